# revision 33
# baseline (speedup 1.0000x reference)
"""Trainium2 Bass kernel for nn_Encoder_Decoder_60146722013205.

Strategy: pure data-parallel over batch (BS=8 -> one batch element per
NeuronCore). Each core runs the full encoder/decoder/generator on its batch
element; no collectives. Activations live transposed in SBUF as
[D(part), T(free)] so weight-stationary matmuls need no transposes.

Key device techniques:
 - all heavy GEMMs in fp8 DoubleRow (PSUM accumulates f32); residual stream
   kept bf16; attention scores/AV in bf16.
 - LayerNorm over the partition axis via bf16 ones-matmul stats + rank-1
   broadcast matmuls + two fused scalar_tensor_tensor passes.
 - softmax without max-subtraction (scores are O(1) by construction);
   scores computed pre-transposed (S^T = K^T.T @ Q^T), denominator obtained
   free by augmenting V with a ones column; per-query normalization via a
   DVE fast-reciprocal on gathered denominator rows + K=1 rank-1 broadcast
   matmuls (no DRAM bounce).
 - elementwise work spread across Act/DVE/Pool engines to keep the PE fed
   (PE p-state drops to half clock on any multi-us stall).
 - log-softmax row sums via activation(Exp) accum_out, final subtract split
   across Vector/Scalar engines in quarters pipelined with the output DMA.
"""

import dataclasses
import math
import os

import ml_dtypes
import numpy as np

import concourse.bass as bass
import concourse.mybir as mybir
import concourse.tile as tile
from concourse.bass_utils import run_bass_kernel_spmd
from concourse.vector_clock import ScopedClock

# ---------------------------------------------------------------------------
# This image's `antenv` package lacks `axon_hooks`, which bass_utils imports
# unconditionally when trace=True under axon. Provide it: a tiny registry plus
# the same ctypes NTFF hook trn_boot would have installed.
# ---------------------------------------------------------------------------
def _ensure_axon_hooks():
    import sys
    import types
    try:
        import antenv.axon_hooks  # noqa: F401
        return
    except ImportError:
        pass
    mod = types.ModuleType("antenv.axon_hooks")
    _hook = [None]
    mod.set_axon_ntff_profile_hook = lambda h: _hook.__setitem__(0, h)
    mod.get_axon_ntff_profile_hook = lambda: _hook[0]
    sys.modules["antenv.axon_hooks"] = mod
    try:
        import antenv
        antenv.axon_hooks = mod
    except ImportError:
        pass
    try:
        from trn_agent_boot.trn_boot import _ntff_profile_via_ctypes
        so = "/opt/axon/libaxon_pjrt.so"
        if os.path.exists(so):
            mod.set_axon_ntff_profile_hook(_ntff_profile_via_ctypes(so))
    except Exception:
        pass


_ensure_axon_hooks()

F32 = mybir.dt.float32
F8 = mybir.dt.float8e4
FP8_SCALE = 32.0
F16 = mybir.dt.float16
BF16 = mybir.dt.bfloat16
AF = mybir.ActivationFunctionType
ALU = mybir.AluOpType
AX = mybir.AxisListType

NL, NH, HD, D, F = 6, 8, 64, 512, 2048
VS = 32000
BS, LS, LT = 8, 512, 256
P = 128
DC = D // P          # 4 chunks of the model dim
FC = F // P          # 16 chunks of the ff dim
EPS = 1e-6
GCH = 1024           # generator vocab chunk (two PSUM banks)
ECH = 4096           # generator exp/accum chunk

# layer-norm index map (32 norms total)
def LN_E1(l): return 2 * l
def LN_E2(l): return 2 * l + 1
LN_ENCN = 12
def LN_D1(l): return 13 + 3 * l
def LN_D2(l): return 14 + 3 * l
def LN_D3(l): return 15 + 3 * l
LN_DECN = 31
N_LN = 32

LAST_RESULTS = None  # BassKernelResults of the most recent run (for test.py)

# ---------------------------------------------------------------------------
# walrus workaround: this toolchain rejects instructions carrying more than
# one semaphore wait ("Too many sync wait commands"). Tile attaches several.
# Split: every instruction keeps 1 wait; extras move to same-engine NoOps
# inserted immediately before it.
# ---------------------------------------------------------------------------
_MAXW = 1
_split_n = [0]


def _drain_and_barrier_split(self, tick_clock, wait_clock):
    nc = self.nc
    carrier = nc.sync.drain()
    wait_clock.add_sem_waits(carrier.ins, ScopedClock({None: tick_clock.global_clock}))
    nc.all_engine_barrier()
    assert self.sems is not None
    popped = nc._tile_sem_poison_stack.pop()
    assert popped is self._sem_poison
    nc.clear_and_free_semaphores(list(self.sems.allocated().values()))
    nc.all_engine_barrier()


tile.TileContext._drain_and_barrier = _drain_and_barrier_split


def _split_waits(nc):
    for f in nc.m.functions:
        for bb in f.blocks:
            insts = list(bb.instructions)
            out = []
            changed = False
            for ins in insts:
                si = ins.sync_info
                if si is not None and len(si.on_wait) > _MAXW:
                    waits = list(si.on_wait)
                    for i in range(_MAXW, len(waits), _MAXW):
                        _split_n[0] += 1
                        n = mybir.InstNoOp(name=f"waitsplit-{_split_n[0]}", ins=[], outs=[])
                        n.engine = ins.engine
                        n.sync_info = mybir.SyncInfo(on_wait=waits[i:i + _MAXW], on_update=[])
                        out.append(n)
                    ins.sync_info = mybir.SyncInfo(on_wait=waits[:_MAXW], on_update=list(si.on_update))
                    changed = True
                out.append(ins)
            if changed:
                bb.instructions = out


# ---------------------------------------------------------------------------
# program builder
# ---------------------------------------------------------------------------
def build_program(use_dec_mask, debug=False, fp8=True, pool=True):
    nc = bass.Bass()
    wdt8 = F8 if fp8 else BF16

    x0t = nc.declare_dram_parameter("x0t", [P, DC, LS], BF16, isOutput=False)
    y0t = nc.declare_dram_parameter("y0t", [P, DC, LT], BF16, isOutput=False)
    w = {}
    for pfx in ("e", "d"):
        w[pfx + "wq"] = nc.declare_dram_parameter(pfx + "wq", [NL, P, DC, D], wdt8, isOutput=False)
        w[pfx + "wk"] = nc.declare_dram_parameter(pfx + "wk", [NL, P, DC, D], wdt8, isOutput=False)
        w[pfx + "wv"] = nc.declare_dram_parameter(pfx + "wv", [NL, P, DC, D], wdt8, isOutput=False)
        w[pfx + "wo"] = nc.declare_dram_parameter(pfx + "wo", [NL, P, DC, D], wdt8, isOutput=False)
        w[pfx + "ff1"] = nc.declare_dram_parameter(pfx + "ff1", [NL, P, DC, F], wdt8, isOutput=False)
        w[pfx + "ff2"] = nc.declare_dram_parameter(pfx + "ff2", [NL, P, FC, D], wdt8, isOutput=False)
    genw = nc.declare_dram_parameter("genw", [P, DC, VS], wdt8, isOutput=False)
    ident_d = nc.declare_dram_parameter("ident", [P, 2, P], BF16, isOutput=False)
    ln_cols_d = nc.declare_dram_parameter("ln_cols", [P, N_LN, 2, DC], F32, isOutput=False)
    ln_rows_d = nc.declare_dram_parameter("ln_rows", [N_LN, 1, D], F16, isOutput=False)
    dmask_d = None
    if use_dec_mask:
        dmask_d = nc.declare_dram_parameter("dmaskt", [P, LT // P, LT], BF16, isOutput=False)

    out_d = nc.declare_dram_parameter("out", [LT, VS], BF16, isOutput=True)
    dbg = {}
    if debug:
        for nm, shp in (("dbg_xenc", [P, DC, LS]), ("dbg_y", [P, DC, LT])):
            dbg[nm] = nc.declare_dram_parameter(nm, shp, F32, isOutput=True)

    with tile.TileContext(nc) as tc:
        _build_body(nc, tc, x0t, y0t, w, genw, ident_d, ln_cols_d, ln_rows_d,
                    dmask_d, out_d, dbg, fp8, pool)
    _split_waits(nc)
    return nc


def _build_body(nc, tc, x0t, y0t, w, genw, ident_d, ln_cols_d, ln_rows_d,
                dmask_d, out_d, dbg, fp8, pool):
    F8A = F8 if fp8 else BF16          # ff/generator/attn activation dtype
    fp8a = fp8                         # fp8 attention projections
    KSTEPA = 2 if fp8a else 1
    PMODEA = mybir.MatmulPerfMode.DoubleRow if fp8a else None
    DSCA = (1.0 / FP8_SCALE) if fp8a else 1.0
    DSC = (1.0 / FP8_SCALE) if fp8 else 1.0
    PMODE = mybir.MatmulPerfMode.DoubleRow if fp8 else None
    KSTEP = 2 if fp8 else 1
    # GPSIMD/Pool rejects tensor ops in this toolchain; keep on DVE
    po_eng = nc.vector
    po_eng2 = nc.vector
    # PSUM->SBUF drains round-robin between the Act and DVE engines
    _alt = [0]

    def drain(dst, src, scale):
        _alt[0] += 1
        if _alt[0] % 2 == 0:
            nc.scalar.activation(out=dst, in_=src, func=AF.Identity,
                                 bias=0.0, scale=scale)
        else:
            nc.vector.tensor_scalar_mul(dst, src, scale)

    def relu_drain(dst, src):
        _alt[0] += 1
        if _alt[0] % 2 == 0:
            nc.scalar.activation(out=dst, in_=src, func=AF.Relu,
                                 bias=0.0, scale=1.0)
        else:
            nc.vector.tensor_scalar_max(dst, src, 0.0)
    from contextlib import ExitStack
    ctx = ExitStack()
    with ctx:
        persist = ctx.enter_context(tc.tile_pool(name="persist", bufs=1))
        rows = ctx.enter_context(tc.tile_pool(name="rows", bufs=1))
        pp = ctx.enter_context(tc.tile_pool(name="pp", bufs=4, space="PSUM"))

        # resident constants
        ln_cols = persist.tile([P, N_LN, 2, DC], F32)
        nc.sync.dma_start(out=ln_cols[:], in_=ln_cols_d[:])
        ones_cB = persist.tile([P, 1], BF16)
        nc.vector.memset(ones_cB[:], 1.0 / D)
        ones_r16 = persist.tile([1, P], F16)
        nc.vector.memset(ones_r16[:], 1.0)
        ones_rB = persist.tile([1, P], BF16)
        nc.vector.memset(ones_rB[:], 1.0)
        ones64 = persist.tile([65, P], BF16)
        nc.vector.memset(ones64[64:65, :], 1.0)
        ones64_16 = persist.tile([65, P], F16)
        nc.vector.memset(ones64_16[64:65, :], 1.0)
        # [1, 64] ones row for per-head K=1 reciprocal broadcasts
        O8 = persist.tile([1, HD], F16)
        nc.vector.memset(O8[:], 1.0)

        def heater(row_ap):
            """Tiny rank-1 matmul that keeps the PE HAM busy during gaps.
            row_ap: a [1, N<=512] SBUF row (f16 or bf16) produced by the op the
            heater should follow; output is garbage and never read."""
            hn = min(row_ap.shape[-1], 512)
            hp = pp.tile([P, 512], F32, tag="ps", name="heat")
            if row_ap.base_partition() == 64:
                ones = (ones64_16 if row_ap.dtype == F16 else ones64)[64:65, :]
            else:
                ones = (ones_r16 if row_ap.dtype == F16 else ones_rB)[:]
            nc.tensor.matmul(hp[:, 0:hn], ones, row_ap[..., 0:hn],
                             start=True, stop=True)

        def f16row(t):
            """view an f32 [1, T] row as [1, T] f16 (garbage values, heater fuel)"""
            return t[:].bitcast(F16)[0:1, 0:min(t.shape[-1], 512)]
        eps_t = persist.tile([P, 1], F32)
        nc.vector.memset(eps_t[:], EPS)

        x = persist.tile([P, DC, LS], BF16)
        nc.sync.dma_start(out=x[:], in_=x0t[:])
        y = persist.tile([P, DC, LT], BF16)
        nc.sync.dma_start(out=y[:], in_=y0t[:])
        zt = persist.tile([P, DC, LS], F8A)  # encoder output, cross K/V source
        # scaled identity matrices: residual folded into the PSUM accumulation
        # (ps += s*x), undone by the drain scale
        ident = persist.tile([P, 2, P], BF16)
        nc.sync.dma_start(out=ident[:], in_=ident_d[:])

        dmask = None
        if dmask_d is not None:
            dmask = persist.tile([P, LT // P, LT], BF16)
            nc.sync.dma_start(out=dmask[:], in_=dmask_d[:])

        cur_apool = None
        cur_rpp = None

        # --------------- helpers ---------------
        def layer_norm(src, T, ln_idx, out_dt=BF16, apool=None, tag="xn"):
            """src: bf16 [P, DC, T] -> normalized [P, DC, T]."""
            grow = apool.tile([1, D], F16, tag="grow", bufs=2)
            nc.sync.dma_start(out=grow[:], in_=ln_rows_d[ln_idx])
            x2 = apool.tile([P, DC, T], BF16, tag="x2", bufs=1)
            po_eng2.tensor_mul(x2[:], src[:], src[:])
            meanp = pp.tile([1, T], F32, tag="ps")
            for kc in range(DC):
                nc.tensor.matmul(meanp[:], ones_cB[:], src[:, kc, :],
                                 start=(kc == 0), stop=(kc == DC - 1))
            esqp = pp.tile([1, T], F32, tag="ps")
            for kc in range(DC):
                nc.tensor.matmul(esqp[:], ones_cB[:], x2[:, kc, :],
                                 start=(kc == 0), stop=(kc == DC - 1))
            mean = rows.tile([1, T], F32, tag="r_mean")
            nc.vector.tensor_copy(mean[:], meanp[:])
            var = rows.tile([1, T], F32, tag="r_var")
            po_eng.scalar_tensor_tensor(out=var[:], in0=mean[:], scalar=-1.0,
                                        in1=mean[:], op0=ALU.mult, op1=ALU.mult)
            nc.vector.tensor_add(var[:], var[:], esqp[:])
            heater(f16row(var))
            lnv = rows.tile([1, T], F32, tag="r_lnv")
            nc.scalar.activation(out=lnv[:], in_=var[:], func=AF.Ln, bias=eps_t[0:1, :], scale=1.0)
            rstd = rows.tile([1, T], F32, tag="r_rstd")
            nc.scalar.activation(out=rstd[:], in_=lnv[:], func=AF.Exp, bias=0.0, scale=-0.5)
            heater(f16row(lnv))
            rstd16 = rows.tile([1, T], F16, tag="r_rstd16")
            po_eng.tensor_copy(rstd16[:], rstd[:])
            mr16 = rows.tile([1, T], F16, tag="r_mr16")
            po_eng.tensor_mul(mr16[:], mean[:], rstd[:])
            brstd = pp.tile([P, T], F32, tag="ps")
            nc.tensor.matmul(brstd[:], ones_r16[:], rstd16[:], start=True, stop=True)
            xn = apool.tile([P, DC, T], out_dt, tag=tag)
            with nc.allow_low_precision(reason="activations in fp8"):
                for c in range(DC):
                    gm = pp.tile([P, T], F32, tag="ps")
                    nc.tensor.matmul(gm[:], grow[0:1, c * P:(c + 1) * P], mr16[:],
                                     start=True, stop=True)
                    u = apool.tile([P, T], F32, tag="u")
                    nc.vector.scalar_tensor_tensor(
                        out=u[:], in0=src[:, c, :], scalar=ln_cols[:, ln_idx, 0, c:c + 1],
                        in1=brstd[:], op0=ALU.mult, op1=ALU.mult)
                    nc.vector.scalar_tensor_tensor(
                        out=xn[:, c, :], in0=u[:], scalar=ln_cols[:, ln_idx, 1, c:c + 1],
                        in1=gm[:], op0=ALU.add, op1=ALU.subtract)
            return xn

        def load_w(dram, l, shape, apool, tag, bufs=2, dt=BF16):
            t = apool.tile(shape, dt, tag=tag, bufs=bufs)
            nc.sync.dma_start(out=t[:], in_=dram[l])
            return t

        def proj_to_rows(wt, src, T, tag="projo"):
            """out[m-chunk] = W.T @ src: returns bf16 [P, DC, T] (Dout on part)."""
            ot = cur_apool.tile([P, DC, T], BF16, tag=tag)
            for m in range(DC):
                ps = pp.tile([P, T], F32, tag="ps")
                for kc in range(0, DC, KSTEPA):
                    nc.tensor.matmul(
                        ps[:],
                        wt[:, kc:kc + KSTEPA, m * P:(m + 1) * P] if fp8a else wt[:, kc, m * P:(m + 1) * P],
                        src[:, kc:kc + KSTEPA, :] if fp8a else src[:, kc, :],
                        start=(kc == 0), stop=(kc == DC - KSTEPA), perf_mode=PMODEA)
                drain(ot[:, m, :], ps[:], DSCA)
            return ot

        def attention(xn_q, src_kv, Tq, Tk, wqt, wkt, wvt, wot, resid, mask=None):
            KT = Tk // P
            qt = proj_to_rows(wqt, xn_q, Tq, tag="projq")
            kt = proj_to_rows(wkt, src_kv, Tk, tag="projk")
            e8 = (mask is None) and fp8          # fp8 expS/V on maskless attns
            edt = F8A if e8 else BF16
            KS = 2 if (e8 and KT >= 2) else 1    # AV DoubleRow over t-pairs
            PMAV = mybir.MatmulPerfMode.DoubleRow if KS == 2 else None
            ONE = FP8_SCALE if fp8a else 1.0     # V kept at raw weight scale
            # V in [token, d] layout, head-parity blocked (host permutes wv
            # cols): parity 0 heads put V at cols 0:HD + denom col HD; parity
            # 1 heads put V at cols HD:P + denom col 0, so their AV outputs
            # land directly on partitions 64..127 (no cross-partition DMA).
            # Unused stationary columns are never zeroed: the garbage output
            # partitions are simply never read.
            vaug = cur_apool.tile([P, KT, 2, NH // 2, P], edt, tag="vaug", bufs=1)
            with nc.allow_low_precision(reason="attn V in fp8"):
                for hh in range(NH // 2):
                    nc.vector.memset(vaug[:, :, 0, hh, HD:HD + 1], ONE)
                    nc.vector.memset(vaug[:, :, 1, hh, 0:1], ONE)
                for t in range(KT):
                    ps = pp.tile([P, D], F32, tag="ps")
                    for kc in range(0, DC, KSTEPA):
                        nc.tensor.matmul(
                            ps[:],
                            src_kv[:, kc:kc + KSTEPA, t * P:(t + 1) * P] if fp8a else src_kv[:, kc, t * P:(t + 1) * P],
                            wvt[:, kc:kc + KSTEPA, :] if fp8a else wvt[:, kc, :],
                            start=(kc == 0), stop=(kc == DC - KSTEPA), perf_mode=PMODEA)
                    psr = ps[:].rearrange("p (r h e) -> p r h e", r=2, h=NH // 2)
                    drain(vaug[:, t, 0, :, 0:HD], psr[:, 0, :, :], 1.0)
                    drain(vaug[:, t, 1, :, HD:P], psr[:, 1, :, :], 1.0)
            # denominator rows: group g lives at partitions 32g..32g+3 so the
            # Act ops on each 4-row group start at an aligned partition base
            osbx = cur_apool.tile([P, NH, Tq], BF16, tag="osbx", bufs=1)
            denb = cur_apool.tile([36, Tq], BF16, tag="denb", bufs=2)
            lnden = cur_apool.tile([36, Tq], F32, tag="lnden", bufs=2)
            rec16 = cur_apool.tile([36, Tq], F16, tag="rec16", bufs=2)
            rec16r = cur_apool.tile([1, NH, Tq], F16, tag="rec16r", bufs=2)
            ztl = cur_apool.tile([P, DC, Tq], F8A, tag="ztl", bufs=1)
            for g in range(2):
                for h in range(4 * g, 4 * g + 4):
                    expS = cur_apool.tile([P, KT, Tq], edt, tag="expS", bufs=2)
                    hb, hc = (h % 2) * HD, h // 2
                    with nc.allow_low_precision(reason="attn weights in fp8"):
                        if Tq > 256:
                            # single-chunk scores: sps fits one PSUM bank,
                            # freeing banks for a deeper main accumulator ring
                            for k2 in range(KT):
                                sps = cur_rpp.tile([P, Tq], F32, tag="sps2", bufs=2)
                                nc.tensor.matmul(sps[:],
                                                 kt[hb:hb + HD, hc, k2 * P:(k2 + 1) * P],
                                                 qt[hb:hb + HD, hc, :], start=True, stop=True)
                                nc.scalar.activation(out=expS[:, k2, :], in_=sps[:],
                                                     func=AF.Exp, bias=0.0, scale=1.0 / math.sqrt(HD))
                        else:
                            for k2 in range(0, KT, 2):
                                sps = cur_rpp.tile([P, 2, Tq], F32, tag="sps2", bufs=2)
                                for t in range(2):
                                    nc.tensor.matmul(sps[:, t, :],
                                                     kt[hb:hb + HD, hc, (k2 + t) * P:(k2 + t + 1) * P],
                                                     qt[hb:hb + HD, hc, :], start=True, stop=True)
                                nc.scalar.activation(out=expS[:, k2:k2 + 2, :], in_=sps[:],
                                                     func=AF.Exp, bias=0.0, scale=1.0 / math.sqrt(HD))
                    if mask is not None:
                        po_eng2.tensor_mul(expS[:], expS[:], mask[:])
                    oaug = cur_rpp.tile([P, Tq], F32, tag="oaug", bufs=2)
                    for ts in range(0, KT, KS):
                        nc.tensor.matmul(oaug[:],
                                         vaug[:, ts:ts + KS, h % 2, h // 2, :] if KS == 2 else vaug[:, ts, h % 2, h // 2, :],
                                         expS[:, ts:ts + KS, :] if KS == 2 else expS[:, ts, :],
                                         start=(ts == 0), stop=(ts == KT - KS),
                                         perf_mode=PMAV)
                    drain(osbx[:, h, :], oaug[:], 1.0)
                    # gather denominator row -> partition 32*(h//4) + h%4
                    dp = HD if h % 2 == 0 else 0
                    tp = 32 * g + h % 4
                    nc.sync.dma_start(out=denb[tp:tp + 1, :], in_=osbx[dp:dp + 1, h, :])
                # per 4-head group: reciprocal chain overlaps the next group
                g4, g32 = 4 * g, 32 * g
                nc.scalar.activation(out=lnden[g32:g32 + 4, :], in_=denb[g32:g32 + 4, :],
                                     func=AF.Ln, bias=eps_t[g32:g32 + 4, :], scale=1.0)
                nc.scalar.activation(out=rec16[g32:g32 + 4, :], in_=lnden[g32:g32 + 4, :],
                                     func=AF.Exp, bias=0.0, scale=-1.0)
                # pack reciprocal rows onto one partition for the broadcasts
                nc.sync.dma_start(out=rec16r[0:1, g4:g4 + 4, :], in_=rec16[g32:g32 + 4, :])
                # keep the PE p-state hot through the reciprocal chain
                heater(osbx[0:1, 4 * g + 3, 0:min(Tq, 512)])
                heater(rec16r[0:1, g4, 0:min(Tq, 512)])
                heater(rec16r[0:1, g4 + 2, 0:min(Tq, 512)])
            with nc.allow_low_precision(reason="normalized attn out in fp8"):
                for c in range(DC):
                    bc = pp.tile([P, Tq], F32, tag="ps")
                    nc.tensor.matmul(bc[0:HD, :], O8[:],
                                     rec16r[0:1, 2 * c, :], start=True, stop=True)
                    nc.tensor.matmul(bc[HD:P, :], O8[:],
                                     rec16r[0:1, 2 * c + 1, :], start=True, stop=True)
                    nc.vector.tensor_mul(ztl[0:HD, c, :], osbx[0:HD, 2 * c, :],
                                         bc[0:HD, :])
                    nc.vector.tensor_mul(ztl[HD:P, c, :], osbx[HD:P, 2 * c + 1, :],
                                         bc[HD:P, :])
            for m in range(DC):
                ps = pp.tile([P, Tq], F32, tag="ps")
                for c in range(0, DC, KSTEPA):
                    nc.tensor.matmul(
                        ps[:],
                        wot[:, c:c + KSTEPA, m * P:(m + 1) * P] if fp8a else wot[:, c, m * P:(m + 1) * P],
                        ztl[:, c:c + KSTEPA, :] if fp8a else ztl[:, c, :],
                        start=(c == 0), stop=False, perf_mode=PMODEA)
                # residual folded in: ps += (1/DSCA)*resid
                nc.tensor.matmul(ps[:], ident[:, 0, :], resid[:, m, :],
                                 start=False, stop=True)
                drain(resid[:, m, :], ps[:], DSCA)

        def ffn(xn, w1t, w2t, T, resid):
            hbf = cur_apool.tile([P, FC, T], F8A, tag="hbf", bufs=1)
            with nc.allow_low_precision(reason="ff hidden in fp8"):
                for fm in range(FC):
                    ps = pp.tile([P, T], F32, tag="ps")
                    for kc in range(0, DC, KSTEP):
                        nc.tensor.matmul(
                            ps[:],
                            w1t[:, kc:kc + KSTEP, fm * P:(fm + 1) * P] if fp8 else w1t[:, kc, fm * P:(fm + 1) * P],
                            xn[:, kc:kc + KSTEP, :] if fp8 else xn[:, kc, :],
                            start=(kc == 0), stop=(kc == DC - KSTEP),
                            perf_mode=PMODE)
                    # raw relu; the *DSC*DSC rescale happens in the ff2 drain
                    relu_drain(hbf[:, fm, :], ps[:])
                for m in range(DC):
                    ps = pp.tile([P, T], F32, tag="ps")
                    for fc in range(0, FC, KSTEP):
                        nc.tensor.matmul(
                            ps[:],
                            w2t[:, fc:fc + KSTEP, m * P:(m + 1) * P] if fp8 else w2t[:, fc, m * P:(m + 1) * P],
                            hbf[:, fc:fc + KSTEP, :] if fp8 else hbf[:, fc, :],
                            start=(fc == 0), stop=False,
                            perf_mode=PMODE)
                    # residual folded in: ps += (1/(DSC*DSC))*resid
                    nc.tensor.matmul(ps[:], ident[:, 1, :], resid[:, m, :],
                                     start=False, stop=True)
                    drain(resid[:, m, :], ps[:], DSC * DSC)

        # --------------- encoder ---------------
        with tc.tile_pool(name="enc_w", bufs=2) as wpool, \
             tc.tile_pool(name="enc_a", bufs=2) as apool, \
             tc.tile_pool(name="enc_p", bufs=2, space="PSUM") as rpp:
            cur_apool = apool
            cur_rpp = rpp
            for l in range(NL):
                wq = load_w(w["ewq"], l, [P, DC, D], wpool, "wq", dt=F8A)
                wk = load_w(w["ewk"], l, [P, DC, D], wpool, "wk", dt=F8A)
                wv = load_w(w["ewv"], l, [P, DC, D], wpool, "wv", dt=F8A)
                wo = load_w(w["ewo"], l, [P, DC, D], wpool, "wo", dt=F8A)
                w1 = load_w(w["eff1"], l, [P, DC, F], wpool, "ff1", bufs=1, dt=F8A)
                w2 = load_w(w["eff2"], l, [P, FC, D], wpool, "ff2", bufs=1, dt=F8A)
                xn1 = layer_norm(x, LS, LN_E1(l), out_dt=F8A, apool=apool)
                attention(xn1, xn1, LS, LS, wq, wk, wv, wo, x, mask=None)
                xn2 = layer_norm(x, LS, LN_E2(l), out_dt=F8A, apool=apool, tag="xn8")
                ffn(xn2, w1, w2, LS, x)
            # final encoder norm -> zt
            zfin = layer_norm(x, LS, LN_ENCN, out_dt=F8A, apool=apool)
            with nc.allow_low_precision(reason="encoder output in fp8"):
                po_eng.tensor_copy(zt[:], zfin[:])
            if dbg:
                nc.sync.dma_start(out=dbg["dbg_xenc"][:], in_=x[:])

        # --------------- decoder ---------------
        with tc.tile_pool(name="dec_w", bufs=2) as wpool, \
             tc.tile_pool(name="dec_a", bufs=2) as apool, \
             tc.tile_pool(name="dec_p", bufs=2, space="PSUM") as rpp:
            cur_apool = apool
            cur_rpp = rpp
            for l in range(NL):
                wq = load_w(w["dwq"], l, [P, DC, D], wpool, "wq", dt=F8A)
                wk = load_w(w["dwk"], l, [P, DC, D], wpool, "wk", dt=F8A)
                wv = load_w(w["dwv"], l, [P, DC, D], wpool, "wv", dt=F8A)
                wo = load_w(w["dwo"], l, [P, DC, D], wpool, "wo", dt=F8A)
                w1 = load_w(w["dff1"], l, [P, DC, F], wpool, "ff1", bufs=1, dt=F8A)
                w2 = load_w(w["dff2"], l, [P, FC, D], wpool, "ff2", bufs=1, dt=F8A)
                yn1 = layer_norm(y, LT, LN_D1(l), out_dt=F8A, apool=apool)
                attention(yn1, yn1, LT, LT, wq, wk, wv, wo, y, mask=dmask)
                yn2 = layer_norm(y, LT, LN_D2(l), out_dt=F8A, apool=apool)
                attention(yn2, zt, LT, LS, wq, wk, wv, wo, y, mask=None)
                yn3 = layer_norm(y, LT, LN_D3(l), out_dt=F8A, apool=apool, tag="xn8")
                ffn(yn3, w1, w2, LT, y)
            if dbg:
                nc.sync.dma_start(out=dbg["dbg_y"][:], in_=y[:])

        # --------------- generator + log-softmax ---------------
        with tc.tile_pool(name="gen_l", bufs=1) as lpool, \
             tc.tile_pool(name="gen_w", bufs=3) as gwpool, \
             tc.tile_pool(name="gen_a", bufs=2) as gapool, \
             tc.tile_pool(name="gen_p", bufs=2, space="PSUM") as gpp:
            cur_apool = gapool
            yf = layer_norm(y, LT, LN_DECN, out_dt=F8A, apool=gapool, tag="xn8")
            logits = [lpool.tile([P, VS], BF16, tag=f"log{t}", name=f"logits{t}")
                      for t in range(LT // P)]
            vchunks = []
            vs = 0
            while vs < VS:
                n = min(GCH, VS - vs)
                vchunks.append((vs, n))
                vs += n
            nech = (VS + ECH - 1) // ECH
            accs = [gapool.tile([P, nech], F32, tag=f"acc{t}", name=f"acc{t}")
                    for t in range(LT // P)]
            exp_done = [0]
            adone = {t: [] for t in range(LT // P)}
            with nc.allow_low_precision(reason="fp8 generator"):
                for j, (vs, n) in enumerate(vchunks):
                    gw = gwpool.tile([P, DC, GCH], F8A, tag="gw")
                    nc.sync.dma_start(out=gw[:, :, 0:n], in_=genw[:, :, vs:vs + n])
                    for t in range(LT // P):
                        gps = gpp.tile([P, 2, GCH // 2], F32, tag="gps")
                        for half in range(2):
                            h0 = half * (GCH // 2)
                            hn = min(GCH // 2, n - h0)
                            if hn <= 0:
                                continue
                            for kc in range(0, DC, KSTEP):
                                nc.tensor.matmul(
                                    gps[:, half, 0:hn],
                                    yf[:, kc:kc + KSTEP, t * P:(t + 1) * P] if fp8 else yf[:, kc, t * P:(t + 1) * P],
                                    gw[:, kc:kc + KSTEP, h0:h0 + hn] if fp8 else gw[:, kc, h0:h0 + hn],
                                    start=(kc == 0), stop=(kc == DC - KSTEP),
                                    perf_mode=PMODE)
                        if n == GCH:
                            dst = logits[t][:, vs:vs + n].rearrange(
                                "p (a b) -> p a b", a=2)
                            src_ap = gps[:]
                        else:
                            dst = logits[t][:, vs:vs + n]
                            src_ap = gps[:, 0, 0:n]
                        if (j + t) % 2 == 0:
                            nc.scalar.activation(out=dst, in_=src_ap,
                                                 func=AF.Identity, bias=0.0, scale=DSC)
                        else:
                            nc.vector.tensor_scalar_mul(dst, src_ap, DSC)
                    # fire exp/accum for any newly completed ECH-sized block
                    done = vs + n
                    while done - exp_done[0] >= ECH or (done == VS and exp_done[0] < VS):
                        es = exp_done[0]
                        n2 = min(ECH, VS - es)
                        for t in range(LT // P):
                            scr = gapool.tile([P, ECH], BF16, tag="scr", bufs=2)
                            nc.scalar.activation(out=scr[:, 0:n2],
                                                 in_=logits[t][:, es:es + n2],
                                                 func=AF.Exp, bias=0.0, scale=1.0,
                                                 accum_out=accs[t][:, len(adone[t]):len(adone[t]) + 1])
                            adone[t].append(es)
                        exp_done[0] += n2
            for t in range(LT // P):
                ssum = gapool.tile([P, 1], F32, tag="ssum")
                nc.vector.reduce_sum(ssum[:], accs[t][:], AX.X)
                logs = gapool.tile([P, 1], F32, tag="logs")
                nc.scalar.activation(out=logs[:], in_=ssum[:], func=AF.Ln,
                                     bias=eps_t[:], scale=1.0)
                nlog = gapool.tile([P, 1], F32, tag="nlog")
                nc.vector.tensor_scalar_mul(nlog[:], logs[:], -1.0)
                Q = VS // 8
                for q in range(8):
                    sl = slice(q * Q, (q + 1) * Q)
                    if q % 2 == 0:
                        nc.vector.tensor_scalar_sub(logits[t][:, sl], logits[t][:, sl], logs[:])
                    else:
                        nc.scalar.activation(out=logits[t][:, sl], in_=logits[t][:, sl],
                                             func=AF.Identity, bias=nlog[:], scale=1.0)
                    nc.sync.dma_start(out=out_d[t * P:(t + 1) * P, sl], in_=logits[t][:, sl])


# ---------------------------------------------------------------------------
# host side
# ---------------------------------------------------------------------------
def _pe_vec(bs):
    pos = np.arange(bs, dtype=np.float32)[:, None]
    div = np.exp(np.arange(0, D, 2, dtype=np.float32) * (-math.log(10000.0) / D))
    ang = pos * div
    return np.stack([np.sin(ang), np.cos(ang)], axis=-1).reshape(bs, D)


def _blk_w(wm, dt=ml_dtypes.bfloat16, scale=1.0):
    """[Din, Dout] -> [P, KC, Dout] with w[p, kc, n] = W[kc*128+p, n]."""
    din, dout = wm.shape
    kc = din // P
    a = wm.astype(np.float32) * scale
    return np.ascontiguousarray(a.reshape(kc, P, dout).transpose(1, 0, 2)).astype(dt)


def _blk_wo(wm, dt=ml_dtypes.bfloat16, scale=1.0):
    """Wo [NH*HD, D] -> head-pair packed [P, DC, D]:
    partition p=(h%2)*64+d, chunk c=h//2 holds Wo row h*64+d."""
    out = np.empty((P, DC, D), dtype=np.float32)
    for h in range(NH):
        rows = wm[h * HD:(h + 1) * HD, :] * scale
        out[(h % 2) * HD:(h % 2) * HD + HD, h // 2, :] = rows
    return np.ascontiguousarray(out).astype(dt)


def _blk_xT(xm, dt=ml_dtypes.bfloat16):
    """[T, D] -> transposed blocked [P, DC, T]."""
    t = xm.T  # [D, T]
    return np.ascontiguousarray(
        t.reshape(DC, P, xm.shape[0]).transpose(1, 0, 2)).astype(dt)


def kernel(**inputs):
    global LAST_RESULTS
    inp = {k: np.asarray(v) for k, v in inputs.items()}

    pe = _pe_vec(BS)
    x0 = inp["src_emb"].astype(np.float32)[inp["src"].astype(np.int64)] + pe[:, None, :]
    y0 = inp["tgt_emb"].astype(np.float32)[inp["tgt"].astype(np.int64)] + pe[:, None, :]

    msk_src = inp["msk_src"]
    msk_tgt = inp["msk_tgt"]
    assert np.all(msk_src != 0), "kernel assumes msk_src has no zeros"
    use_dec_mask = not np.all(msk_tgt != 0)

    # shared (replicated) weight tensors
    fp8 = bool(int(os.environ.get("KERNEL_FP8", "1")))
    pool = bool(int(os.environ.get("KERNEL_POOL", "1")))
    w8dt = ml_dtypes.float8_e4m3 if fp8 else ml_dtypes.bfloat16
    w8scale = FP8_SCALE if fp8 else 1.0
    shared = {}
    # wv output columns permuted head-parity-blocked: [0,2,4,6,1,3,5,7]
    vperm = np.concatenate([np.arange(h * HD, (h + 1) * HD)
                            for h in (0, 2, 4, 6, 1, 3, 5, 7)])
    for pfx in ("e", "d"):
        for nm in ("wq", "wk"):
            shared[pfx + nm] = np.stack([
                _blk_w(inp[pfx + nm + "_w"][l], dt=w8dt, scale=w8scale) for l in range(NL)])
        shared[pfx + "wv"] = np.stack([
            _blk_w(inp[pfx + "wv_w"][l][:, vperm], dt=w8dt, scale=w8scale)
            for l in range(NL)])
        for nm in ("ff1", "ff2"):
            shared[pfx + nm] = np.stack([
                _blk_w(inp[pfx + nm + "_w"][l], dt=w8dt, scale=w8scale) for l in range(NL)])
        shared[pfx + "wo"] = np.stack([
            _blk_wo(inp[pfx + "wo_w"][l], dt=w8dt, scale=w8scale) for l in range(NL)])
    shared["genw"] = _blk_w(inp["gen_w"], dt=w8dt, scale=w8scale)
    # scaled identities for residual-in-PSUM accumulation: [P, 2, P]
    eye = np.eye(P, dtype=np.float32)
    shared["ident"] = np.ascontiguousarray(np.stack(
        [eye * w8scale, eye * w8scale * w8scale],
        axis=1)).astype(ml_dtypes.bfloat16)

    for pfx in ("e", "d"):
        for nm in ("wq_b", "wk_b", "wv_b", "wo_b", "ff1_b", "ff2_b"):
            assert np.all(inp[pfx + nm] == 0), f"nonzero bias {pfx+nm} unsupported fast path"
    assert np.all(inp["gen_b"] == 0)

    # layer-norm params: ln_cols [P, 32, 2, DC], ln_rows [1, 32, D] fp16
    g_all = np.zeros((N_LN, D), np.float32)
    b_all = np.zeros((N_LN, D), np.float32)
    for l in range(NL):
        g_all[LN_E1(l)] = inp["eln1_g"][l]; b_all[LN_E1(l)] = inp["eln1_b"][l]
        g_all[LN_E2(l)] = inp["eln2_g"][l]; b_all[LN_E2(l)] = inp["eln2_b"][l]
        g_all[LN_D1(l)] = inp["dln1_g"][l]; b_all[LN_D1(l)] = inp["dln1_b"][l]
        g_all[LN_D2(l)] = inp["dln2_g"][l]; b_all[LN_D2(l)] = inp["dln2_b"][l]
        g_all[LN_D3(l)] = inp["dln3_g"][l]; b_all[LN_D3(l)] = inp["dln3_b"][l]
    g_all[LN_ENCN] = inp["encn_g"]; b_all[LN_ENCN] = inp["encn_b"]
    g_all[LN_DECN] = inp["decn_g"]; b_all[LN_DECN] = inp["decn_b"]
    ln_cols = np.stack([g_all, b_all], axis=1)          # [32, 2, D]
    ln_cols = ln_cols.reshape(N_LN, 2, DC, P).transpose(3, 0, 1, 2)  # [P, 32, 2, DC]
    shared["ln_cols"] = np.ascontiguousarray(ln_cols).astype(np.float32)
    shared["ln_rows"] = g_all.reshape(N_LN, 1, D).astype(np.float16)

    nc = build_program(use_dec_mask, debug=bool(int(os.environ.get("KERNEL_DEBUG", "0"))),
                       fp8=fp8, pool=pool)

    in_maps = []
    for b in range(BS):
        m = dict(shared)
        m["x0t"] = _blk_xT(x0[b])
        m["y0t"] = _blk_xT(y0[b])
        if use_dec_mask:
            mk = (msk_tgt[b].T != 0).astype(np.float32)  # [k, q]
            m["dmaskt"] = np.ascontiguousarray(
                mk.reshape(LT // P, P, LT).transpose(1, 0, 2)).astype(ml_dtypes.bfloat16)
        in_maps.append(m)

    res = run_bass_kernel_spmd(nc, in_maps, list(range(BS)))
    LAST_RESULTS = res
    out = np.stack([res.results[b]["out"].astype(np.float32) for b in range(BS)])
    return out


# revision 38
# speedup vs baseline: 1.0403x; 1.0403x over previous
"""Trainium2 Bass kernel for nn_Encoder_Decoder_60146722013205.

Strategy: pure data-parallel over batch (BS=8 -> one batch element per
NeuronCore). Each core runs the full encoder/decoder/generator on its batch
element; no collectives. Activations live transposed in SBUF as
[D(part), T(free)] so weight-stationary matmuls need no transposes.

Key device techniques:
 - all heavy GEMMs in fp8 DoubleRow (PSUM accumulates f32); residual stream
   kept bf16; attention scores/AV in bf16.
 - LayerNorm over the partition axis via bf16 ones-matmul stats + rank-1
   broadcast matmuls + two fused scalar_tensor_tensor passes.
 - softmax without max-subtraction (scores are O(1) by construction);
   scores computed pre-transposed (S^T = K^T.T @ Q^T), denominator obtained
   free by augmenting V with a ones column; per-query normalization via a
   DVE fast-reciprocal on gathered denominator rows + K=1 rank-1 broadcast
   matmuls (no DRAM bounce).
 - elementwise work spread across Act/DVE/Pool engines to keep the PE fed
   (PE p-state drops to half clock on any multi-us stall).
 - log-softmax row sums via activation(Exp) accum_out, final subtract split
   across Vector/Scalar engines in quarters pipelined with the output DMA.
"""

import dataclasses
import math
import os

import ml_dtypes
import numpy as np

import concourse.bass as bass
import concourse.mybir as mybir
import concourse.tile as tile
from concourse.bass_utils import run_bass_kernel_spmd
from concourse.vector_clock import ScopedClock

# ---------------------------------------------------------------------------
# This image's `antenv` package lacks `axon_hooks`, which bass_utils imports
# unconditionally when trace=True under axon. Provide it: a tiny registry plus
# the same ctypes NTFF hook trn_boot would have installed.
# ---------------------------------------------------------------------------
def _ensure_axon_hooks():
    import sys
    import types
    try:
        import antenv.axon_hooks  # noqa: F401
        return
    except ImportError:
        pass
    mod = types.ModuleType("antenv.axon_hooks")
    _hook = [None]
    mod.set_axon_ntff_profile_hook = lambda h: _hook.__setitem__(0, h)
    mod.get_axon_ntff_profile_hook = lambda: _hook[0]
    sys.modules["antenv.axon_hooks"] = mod
    try:
        import antenv
        antenv.axon_hooks = mod
    except ImportError:
        pass
    try:
        from trn_agent_boot.trn_boot import _ntff_profile_via_ctypes
        so = "/opt/axon/libaxon_pjrt.so"
        if os.path.exists(so):
            mod.set_axon_ntff_profile_hook(_ntff_profile_via_ctypes(so))
    except Exception:
        pass


_ensure_axon_hooks()

F32 = mybir.dt.float32
F8 = mybir.dt.float8e4
FP8_SCALE = 32.0
F16 = mybir.dt.float16
BF16 = mybir.dt.bfloat16
AF = mybir.ActivationFunctionType
ALU = mybir.AluOpType
AX = mybir.AxisListType

NL, NH, HD, D, F = 6, 8, 64, 512, 2048
VS = 32000
BS, LS, LT = 8, 512, 256
P = 128
DC = D // P          # 4 chunks of the model dim
FC = F // P          # 16 chunks of the ff dim
EPS = 1e-6
GCH = 1024           # generator vocab chunk (two PSUM banks)
ECH = 4096           # generator exp/accum chunk

# layer-norm index map (32 norms total)
def LN_E1(l): return 2 * l
def LN_E2(l): return 2 * l + 1
LN_ENCN = 12
def LN_D1(l): return 13 + 3 * l
def LN_D2(l): return 14 + 3 * l
def LN_D3(l): return 15 + 3 * l
LN_DECN = 31
N_LN = 32

LAST_RESULTS = None  # BassKernelResults of the most recent run (for test.py)

# ---------------------------------------------------------------------------
# walrus workaround: this toolchain rejects instructions carrying more than
# one semaphore wait ("Too many sync wait commands"). Tile attaches several.
# Split: every instruction keeps 1 wait; extras move to same-engine NoOps
# inserted immediately before it.
# ---------------------------------------------------------------------------
_MAXW = 1
_split_n = [0]


def _drain_and_barrier_split(self, tick_clock, wait_clock):
    nc = self.nc
    carrier = nc.sync.drain()
    wait_clock.add_sem_waits(carrier.ins, ScopedClock({None: tick_clock.global_clock}))
    nc.all_engine_barrier()
    assert self.sems is not None
    popped = nc._tile_sem_poison_stack.pop()
    assert popped is self._sem_poison
    nc.clear_and_free_semaphores(list(self.sems.allocated().values()))
    nc.all_engine_barrier()


tile.TileContext._drain_and_barrier = _drain_and_barrier_split


def _split_waits(nc):
    for f in nc.m.functions:
        for bb in f.blocks:
            insts = list(bb.instructions)
            out = []
            changed = False
            for ins in insts:
                si = ins.sync_info
                if si is not None and len(si.on_wait) > _MAXW:
                    waits = list(si.on_wait)
                    for i in range(_MAXW, len(waits), _MAXW):
                        _split_n[0] += 1
                        n = mybir.InstNoOp(name=f"waitsplit-{_split_n[0]}", ins=[], outs=[])
                        n.engine = ins.engine
                        n.sync_info = mybir.SyncInfo(on_wait=waits[i:i + _MAXW], on_update=[])
                        out.append(n)
                    ins.sync_info = mybir.SyncInfo(on_wait=waits[:_MAXW], on_update=list(si.on_update))
                    changed = True
                out.append(ins)
            if changed:
                bb.instructions = out


# ---------------------------------------------------------------------------
# program builder
# ---------------------------------------------------------------------------
def build_program(use_dec_mask, debug=False, fp8=True, pool=True):
    nc = bass.Bass()
    wdt8 = F8 if fp8 else BF16

    x0t = nc.declare_dram_parameter("x0t", [P, DC, LS], BF16, isOutput=False)
    y0t = nc.declare_dram_parameter("y0t", [P, DC, LT], BF16, isOutput=False)
    w = {}
    for pfx in ("e", "d"):
        w[pfx + "wq"] = nc.declare_dram_parameter(pfx + "wq", [NL, P, DC, D], wdt8, isOutput=False)
        w[pfx + "wk"] = nc.declare_dram_parameter(pfx + "wk", [NL, P, DC, D], wdt8, isOutput=False)
        w[pfx + "wv"] = nc.declare_dram_parameter(pfx + "wv", [NL, P, DC, D], wdt8, isOutput=False)
        w[pfx + "wo"] = nc.declare_dram_parameter(pfx + "wo", [NL, P, DC, D], wdt8, isOutput=False)
        w[pfx + "ff1"] = nc.declare_dram_parameter(pfx + "ff1", [NL, P, DC, F], wdt8, isOutput=False)
        w[pfx + "ff2"] = nc.declare_dram_parameter(pfx + "ff2", [NL, P, FC, D], wdt8, isOutput=False)
    genw = nc.declare_dram_parameter("genw", [P, DC, VS], wdt8, isOutput=False)
    ident_d = nc.declare_dram_parameter("ident", [P, 2, P], BF16, isOutput=False)
    ln_cols_d = nc.declare_dram_parameter("ln_cols", [P, N_LN, 2, DC], F32, isOutput=False)
    ln_rows_d = nc.declare_dram_parameter("ln_rows", [N_LN, 1, D], F16, isOutput=False)
    dmask_d = None
    if use_dec_mask:
        dmask_d = nc.declare_dram_parameter("dmaskt", [P, LT // P, LT], BF16, isOutput=False)

    out_d = nc.declare_dram_parameter("out", [LT, VS], BF16, isOutput=True)
    dbg = {}
    if debug:
        for nm, shp in (("dbg_xenc", [P, DC, LS]), ("dbg_y", [P, DC, LT])):
            dbg[nm] = nc.declare_dram_parameter(nm, shp, F32, isOutput=True)

    with tile.TileContext(nc) as tc:
        _build_body(nc, tc, x0t, y0t, w, genw, ident_d, ln_cols_d, ln_rows_d,
                    dmask_d, out_d, dbg, fp8, pool)
    _split_waits(nc)
    return nc


def _build_body(nc, tc, x0t, y0t, w, genw, ident_d, ln_cols_d, ln_rows_d,
                dmask_d, out_d, dbg, fp8, pool):
    F8A = F8 if fp8 else BF16          # ff/generator/attn activation dtype
    fp8a = fp8                         # fp8 attention projections
    KSTEPA = 2 if fp8a else 1
    PMODEA = mybir.MatmulPerfMode.DoubleRow if fp8a else None
    DSCA = (1.0 / FP8_SCALE) if fp8a else 1.0
    DSC = (1.0 / FP8_SCALE) if fp8 else 1.0
    PMODE = mybir.MatmulPerfMode.DoubleRow if fp8 else None
    KSTEP = 2 if fp8 else 1
    # GPSIMD/Pool rejects tensor ops in this toolchain; keep on DVE
    po_eng = nc.vector
    po_eng2 = nc.vector
    # PSUM->SBUF drains round-robin between the Act and DVE engines
    _alt = [0]

    def drain(dst, src, scale):
        _alt[0] += 1
        if _alt[0] % 2 == 0:
            nc.scalar.activation(out=dst, in_=src, func=AF.Identity,
                                 bias=0.0, scale=scale)
        else:
            nc.vector.tensor_scalar_mul(dst, src, scale)

    def relu_drain(dst, src):
        _alt[0] += 1
        if _alt[0] % 2 == 0:
            nc.scalar.activation(out=dst, in_=src, func=AF.Relu,
                                 bias=0.0, scale=1.0)
        else:
            nc.vector.tensor_scalar_max(dst, src, 0.0)
    from contextlib import ExitStack
    ctx = ExitStack()
    with ctx:
        persist = ctx.enter_context(tc.tile_pool(name="persist", bufs=1))
        rows = ctx.enter_context(tc.tile_pool(name="rows", bufs=1))
        pp = ctx.enter_context(tc.tile_pool(name="pp", bufs=4, space="PSUM"))

        # resident constants
        ln_cols = persist.tile([P, N_LN, 2, DC], F32)
        nc.sync.dma_start(out=ln_cols[:], in_=ln_cols_d[:])
        ones_cB = persist.tile([P, 1], BF16)
        nc.vector.memset(ones_cB[:], 1.0 / D)
        ones_c8 = persist.tile([P, DC, 1], F8A)
        with nc.allow_low_precision(reason="stats ones in fp8"):
            nc.vector.memset(ones_c8[:], 1.0 / D)
        ones_r16 = persist.tile([1, P], F16)
        nc.vector.memset(ones_r16[:], 1.0)
        ones_rB = persist.tile([1, P], BF16)
        nc.vector.memset(ones_rB[:], 1.0)
        ones64 = persist.tile([65, P], BF16)
        nc.vector.memset(ones64[64:65, :], 1.0)
        ones64_16 = persist.tile([65, P], F16)
        nc.vector.memset(ones64_16[64:65, :], 1.0)
        # [1, 64] ones row for per-head K=1 reciprocal broadcasts
        O8 = persist.tile([1, HD], F16)
        nc.vector.memset(O8[:], 1.0)

        def heater(row_ap):
            """Tiny rank-1 matmul that keeps the PE HAM busy during gaps.
            row_ap: a [1, N<=512] SBUF row (f16 or bf16) produced by the op the
            heater should follow; output is garbage and never read."""
            hn = min(row_ap.shape[-1], 512)
            hp = pp.tile([P, 512], F32, tag="ps", name="heat")
            if row_ap.base_partition() == 64:
                ones = (ones64_16 if row_ap.dtype == F16 else ones64)[64:65, :]
            else:
                ones = (ones_r16 if row_ap.dtype == F16 else ones_rB)[:]
            nc.tensor.matmul(hp[:, 0:hn], ones, row_ap[..., 0:hn],
                             start=True, stop=True)

        def f16row(t):
            """view an f32 [1, T] row as [1, T] f16 (garbage values, heater fuel)"""
            return t[:].bitcast(F16)[0:1, 0:min(t.shape[-1], 512)]
        eps_t = persist.tile([P, 1], F32)
        nc.vector.memset(eps_t[:], EPS)

        x = persist.tile([P, DC, LS], BF16)
        nc.sync.dma_start(out=x[:], in_=x0t[:])
        y = persist.tile([P, DC, LT], BF16)
        nc.sync.dma_start(out=y[:], in_=y0t[:])
        zt = persist.tile([P, DC, LS], F8A)  # encoder output, cross K/V source
        # scaled identity matrices: residual folded into the PSUM accumulation
        # (ps += s*x), undone by the drain scale
        ident = persist.tile([P, 2, P], BF16)
        nc.sync.dma_start(out=ident[:], in_=ident_d[:])

        dmask = None
        if dmask_d is not None:
            dmask = persist.tile([P, LT // P, LT], BF16)
            nc.sync.dma_start(out=dmask[:], in_=dmask_d[:])

        cur_apool = None
        cur_rpp = None

        # --------------- helpers ---------------
        def layer_norm(src, T, ln_idx, out_dt=BF16, apool=None, tag="xn"):
            """src: bf16 [P, DC, T] -> normalized [P, DC, T]."""
            grow = apool.tile([1, D], F16, tag="grow", bufs=2)
            nc.sync.dma_start(out=grow[:], in_=ln_rows_d[ln_idx])
            x2 = apool.tile([P, DC, T], BF16, tag="x2", bufs=1)
            po_eng2.tensor_mul(x2[:], src[:], src[:])
            meanp = pp.tile([1, T], F32, tag="ps")
            for kc in range(DC):
                nc.tensor.matmul(meanp[:], ones_cB[:], src[:, kc, :],
                                 start=(kc == 0), stop=(kc == DC - 1))
            esqp = pp.tile([1, T], F32, tag="ps")
            for kc in range(DC):
                nc.tensor.matmul(esqp[:], ones_cB[:], x2[:, kc, :],
                                 start=(kc == 0), stop=(kc == DC - 1))
            mean = rows.tile([1, T], F32, tag="r_mean")
            nc.vector.tensor_copy(mean[:], meanp[:])
            var = rows.tile([1, T], F32, tag="r_var")
            po_eng.scalar_tensor_tensor(out=var[:], in0=mean[:], scalar=-1.0,
                                        in1=mean[:], op0=ALU.mult, op1=ALU.mult)
            nc.vector.tensor_add(var[:], var[:], esqp[:])
            heater(f16row(var))
            lnv = rows.tile([1, T], F32, tag="r_lnv")
            nc.scalar.activation(out=lnv[:], in_=var[:], func=AF.Ln, bias=eps_t[0:1, :], scale=1.0)
            rstd16 = rows.tile([1, T], F16, tag="r_rstd16")
            nc.scalar.activation(out=rstd16[:], in_=lnv[:], func=AF.Exp, bias=0.0, scale=-0.5)
            heater(f16row(lnv))
            mr16 = rows.tile([1, T], F16, tag="r_mr16")
            po_eng.tensor_mul(mr16[:], mean[:], rstd16[:])
            brstd = pp.tile([P, T], F32, tag="ps")
            nc.tensor.matmul(brstd[:], ones_r16[:], rstd16[:], start=True, stop=True)
            xn = apool.tile([P, DC, T], out_dt, tag=tag)
            with nc.allow_low_precision(reason="activations in fp8"):
                for c in range(DC):
                    gm = pp.tile([P, T], F32, tag="ps")
                    nc.tensor.matmul(gm[:], grow[0:1, c * P:(c + 1) * P], mr16[:],
                                     start=True, stop=True)
                    u = apool.tile([P, T], F32, tag="u")
                    nc.vector.scalar_tensor_tensor(
                        out=u[:], in0=src[:, c, :], scalar=ln_cols[:, ln_idx, 0, c:c + 1],
                        in1=brstd[:], op0=ALU.mult, op1=ALU.mult)
                    nc.vector.scalar_tensor_tensor(
                        out=xn[:, c, :], in0=u[:], scalar=ln_cols[:, ln_idx, 1, c:c + 1],
                        in1=gm[:], op0=ALU.add, op1=ALU.subtract)
            return xn

        def load_w(dram, l, shape, apool, tag, bufs=2, dt=BF16):
            t = apool.tile(shape, dt, tag=tag, bufs=bufs)
            nc.sync.dma_start(out=t[:], in_=dram[l])
            return t

        def proj_to_rows(wt, src, T, tag="projo"):
            """out[m-chunk] = W.T @ src: returns bf16 [P, DC, T] (Dout on part)."""
            ot = cur_apool.tile([P, DC, T], BF16, tag=tag)
            for m in range(DC):
                ps = pp.tile([P, T], F32, tag="ps")
                for kc in range(0, DC, KSTEPA):
                    nc.tensor.matmul(
                        ps[:],
                        wt[:, kc:kc + KSTEPA, m * P:(m + 1) * P] if fp8a else wt[:, kc, m * P:(m + 1) * P],
                        src[:, kc:kc + KSTEPA, :] if fp8a else src[:, kc, :],
                        start=(kc == 0), stop=(kc == DC - KSTEPA), perf_mode=PMODEA)
                drain(ot[:, m, :], ps[:], DSCA)
            return ot

        def attention(xn_q, src_kv, Tq, Tk, wqt, wkt, wvt, wot, resid, mask=None):
            KT = Tk // P
            qt = proj_to_rows(wqt, xn_q, Tq, tag="projq")
            kt = proj_to_rows(wkt, src_kv, Tk, tag="projk")
            e8 = (mask is None) and fp8          # fp8 expS/V on maskless attns
            edt = F8A if e8 else BF16
            KS = 2 if (e8 and KT >= 2) else 1    # AV DoubleRow over t-pairs
            PMAV = mybir.MatmulPerfMode.DoubleRow if KS == 2 else None
            ONE = FP8_SCALE if fp8a else 1.0     # V kept at raw weight scale
            # V in [token, d] layout, head-parity blocked (host permutes wv
            # cols): parity 0 heads put V at cols 0:HD + denom col HD; parity
            # 1 heads put V at cols HD:P + denom col 0, so their AV outputs
            # land directly on partitions 64..127 (no cross-partition DMA).
            # Unused stationary columns are never zeroed: the garbage output
            # partitions are simply never read.
            vaug = cur_apool.tile([P, KT, 2, NH // 2, P], edt, tag="vaug", bufs=1)
            with nc.allow_low_precision(reason="attn V in fp8"):
                for hh in range(NH // 2):
                    nc.vector.memset(vaug[:, :, 0, hh, HD:HD + 1], ONE)
                    nc.vector.memset(vaug[:, :, 1, hh, 0:1], ONE)
                for t in range(KT):
                    ps = pp.tile([P, D], F32, tag="ps")
                    for kc in range(0, DC, KSTEPA):
                        nc.tensor.matmul(
                            ps[:],
                            src_kv[:, kc:kc + KSTEPA, t * P:(t + 1) * P] if fp8a else src_kv[:, kc, t * P:(t + 1) * P],
                            wvt[:, kc:kc + KSTEPA, :] if fp8a else wvt[:, kc, :],
                            start=(kc == 0), stop=(kc == DC - KSTEPA), perf_mode=PMODEA)
                    psr = ps[:].rearrange("p (r h e) -> p r h e", r=2, h=NH // 2)
                    drain(vaug[:, t, 0, :, 0:HD], psr[:, 0, :, :], 1.0)
                    drain(vaug[:, t, 1, :, HD:P], psr[:, 1, :, :], 1.0)
            # denominator rows: group g lives at partitions 32g..32g+3 so the
            # Act ops on each 4-row group start at an aligned partition base
            osbx = cur_apool.tile([P, NH, Tq], BF16, tag="osbx", bufs=1)
            denb = cur_apool.tile([36, Tq], BF16, tag="denb", bufs=2)
            lnden = cur_apool.tile([36, Tq], F32, tag="lnden", bufs=2)
            rec16 = cur_apool.tile([36, Tq], F16, tag="rec16", bufs=2)
            rec16r = cur_apool.tile([1, NH, Tq], F16, tag="rec16r", bufs=2)
            ztl = cur_apool.tile([P, DC, Tq], F8A, tag="ztl", bufs=1)
            for g in range(2):
                for h in range(4 * g, 4 * g + 4):
                    expS = cur_apool.tile([P, KT, Tq], edt, tag="expS", bufs=2)
                    hb, hc = (h % 2) * HD, h // 2
                    with nc.allow_low_precision(reason="attn weights in fp8"):
                        if Tq > 256:
                            # single-chunk scores: sps fits one PSUM bank,
                            # freeing banks for a deeper main accumulator ring
                            for k2 in range(KT):
                                sps = cur_rpp.tile([P, Tq], F32, tag="sps2", bufs=2)
                                nc.tensor.matmul(sps[:],
                                                 kt[hb:hb + HD, hc, k2 * P:(k2 + 1) * P],
                                                 qt[hb:hb + HD, hc, :], start=True, stop=True)
                                nc.scalar.activation(out=expS[:, k2, :], in_=sps[:],
                                                     func=AF.Exp, bias=0.0, scale=1.0 / math.sqrt(HD))
                        else:
                            for k2 in range(0, KT, 2):
                                sps = cur_rpp.tile([P, 2, Tq], F32, tag="sps2", bufs=2)
                                for t in range(2):
                                    nc.tensor.matmul(sps[:, t, :],
                                                     kt[hb:hb + HD, hc, (k2 + t) * P:(k2 + t + 1) * P],
                                                     qt[hb:hb + HD, hc, :], start=True, stop=True)
                                nc.scalar.activation(out=expS[:, k2:k2 + 2, :], in_=sps[:],
                                                     func=AF.Exp, bias=0.0, scale=1.0 / math.sqrt(HD))
                    if mask is not None:
                        po_eng2.tensor_mul(expS[:], expS[:], mask[:])
                    oaug = cur_rpp.tile([P, Tq], F32, tag="oaug", bufs=2)
                    for ts in range(0, KT, KS):
                        nc.tensor.matmul(oaug[:],
                                         vaug[:, ts:ts + KS, h % 2, h // 2, :] if KS == 2 else vaug[:, ts, h % 2, h // 2, :],
                                         expS[:, ts:ts + KS, :] if KS == 2 else expS[:, ts, :],
                                         start=(ts == 0), stop=(ts == KT - KS),
                                         perf_mode=PMAV)
                    drain(osbx[:, h, :], oaug[:], 1.0)
                    # gather denominator row -> partition 32*(h//4) + h%4
                    dp = HD if h % 2 == 0 else 0
                    tp = 32 * g + h % 4
                    nc.sync.dma_start(out=denb[tp:tp + 1, :], in_=osbx[dp:dp + 1, h, :])
                # per 4-head group: reciprocal chain overlaps the next group
                g4, g32 = 4 * g, 32 * g
                nc.scalar.activation(out=lnden[g32:g32 + 4, :], in_=denb[g32:g32 + 4, :],
                                     func=AF.Ln, bias=eps_t[g32:g32 + 4, :], scale=1.0)
                nc.scalar.activation(out=rec16[g32:g32 + 4, :], in_=lnden[g32:g32 + 4, :],
                                     func=AF.Exp, bias=0.0, scale=-1.0)
                # pack reciprocal rows onto one partition for the broadcasts
                nc.sync.dma_start(out=rec16r[0:1, g4:g4 + 4, :], in_=rec16[g32:g32 + 4, :])

            with nc.allow_low_precision(reason="normalized attn out in fp8"):
                for c in range(DC):
                    bc = pp.tile([P, Tq], F32, tag="ps")
                    nc.tensor.matmul(bc[0:HD, :], O8[:],
                                     rec16r[0:1, 2 * c, :], start=True, stop=True)
                    nc.tensor.matmul(bc[HD:P, :], O8[:],
                                     rec16r[0:1, 2 * c + 1, :], start=True, stop=True)
                    nc.vector.tensor_mul(ztl[0:HD, c, :], osbx[0:HD, 2 * c, :],
                                         bc[0:HD, :])
                    nc.vector.tensor_mul(ztl[HD:P, c, :], osbx[HD:P, 2 * c + 1, :],
                                         bc[HD:P, :])
            for m in range(DC):
                ps = pp.tile([P, Tq], F32, tag="ps")
                for c in range(0, DC, KSTEPA):
                    nc.tensor.matmul(
                        ps[:],
                        wot[:, c:c + KSTEPA, m * P:(m + 1) * P] if fp8a else wot[:, c, m * P:(m + 1) * P],
                        ztl[:, c:c + KSTEPA, :] if fp8a else ztl[:, c, :],
                        start=(c == 0), stop=False, perf_mode=PMODEA)
                # residual folded in: ps += (1/DSCA)*resid
                nc.tensor.matmul(ps[:], ident[:, 0, :], resid[:, m, :],
                                 start=False, stop=True)
                drain(resid[:, m, :], ps[:], DSCA)

        def ffn(xn, w1t, w2t, T, resid):
            hbf = cur_apool.tile([P, FC, T], F8A, tag="hbf", bufs=1)
            with nc.allow_low_precision(reason="ff hidden in fp8"):
                for fm in range(FC):
                    ps = pp.tile([P, T], F32, tag="ps")
                    for kc in range(0, DC, KSTEP):
                        nc.tensor.matmul(
                            ps[:],
                            w1t[:, kc:kc + KSTEP, fm * P:(fm + 1) * P] if fp8 else w1t[:, kc, fm * P:(fm + 1) * P],
                            xn[:, kc:kc + KSTEP, :] if fp8 else xn[:, kc, :],
                            start=(kc == 0), stop=(kc == DC - KSTEP),
                            perf_mode=PMODE)
                    # raw relu; the *DSC*DSC rescale happens in the ff2 drain
                    relu_drain(hbf[:, fm, :], ps[:])
                for m in range(DC):
                    ps = pp.tile([P, T], F32, tag="ps")
                    for fc in range(0, FC, KSTEP):
                        nc.tensor.matmul(
                            ps[:],
                            w2t[:, fc:fc + KSTEP, m * P:(m + 1) * P] if fp8 else w2t[:, fc, m * P:(m + 1) * P],
                            hbf[:, fc:fc + KSTEP, :] if fp8 else hbf[:, fc, :],
                            start=(fc == 0), stop=False,
                            perf_mode=PMODE)
                    # residual folded in: ps += (1/(DSC*DSC))*resid
                    nc.tensor.matmul(ps[:], ident[:, 1, :], resid[:, m, :],
                                     start=False, stop=True)
                    drain(resid[:, m, :], ps[:], DSC * DSC)

        # --------------- encoder ---------------
        with tc.tile_pool(name="enc_w", bufs=2) as wpool, \
             tc.tile_pool(name="enc_a", bufs=2) as apool, \
             tc.tile_pool(name="enc_p", bufs=2, space="PSUM") as rpp:
            cur_apool = apool
            cur_rpp = rpp
            for l in range(NL):
                wq = load_w(w["ewq"], l, [P, DC, D], wpool, "wq", dt=F8A)
                wk = load_w(w["ewk"], l, [P, DC, D], wpool, "wk", dt=F8A)
                wv = load_w(w["ewv"], l, [P, DC, D], wpool, "wv", dt=F8A)
                wo = load_w(w["ewo"], l, [P, DC, D], wpool, "wo", dt=F8A)
                w1 = load_w(w["eff1"], l, [P, DC, F], wpool, "ff1", bufs=1, dt=F8A)
                w2 = load_w(w["eff2"], l, [P, FC, D], wpool, "ff2", bufs=1, dt=F8A)
                xn1 = layer_norm(x, LS, LN_E1(l), out_dt=F8A, apool=apool)
                attention(xn1, xn1, LS, LS, wq, wk, wv, wo, x, mask=None)
                xn2 = layer_norm(x, LS, LN_E2(l), out_dt=F8A, apool=apool, tag="xn8")
                ffn(xn2, w1, w2, LS, x)
            # final encoder norm -> zt
            zfin = layer_norm(x, LS, LN_ENCN, out_dt=F8A, apool=apool)
            with nc.allow_low_precision(reason="encoder output in fp8"):
                po_eng.tensor_copy(zt[:], zfin[:])
            if dbg:
                nc.sync.dma_start(out=dbg["dbg_xenc"][:], in_=x[:])

        # --------------- decoder ---------------
        with tc.tile_pool(name="dec_w", bufs=2) as wpool, \
             tc.tile_pool(name="dec_a", bufs=2) as apool, \
             tc.tile_pool(name="dec_p", bufs=2, space="PSUM") as rpp:
            cur_apool = apool
            cur_rpp = rpp
            for l in range(NL):
                wq = load_w(w["dwq"], l, [P, DC, D], wpool, "wq", dt=F8A)
                wk = load_w(w["dwk"], l, [P, DC, D], wpool, "wk", dt=F8A)
                wv = load_w(w["dwv"], l, [P, DC, D], wpool, "wv", dt=F8A)
                wo = load_w(w["dwo"], l, [P, DC, D], wpool, "wo", dt=F8A)
                w1 = load_w(w["dff1"], l, [P, DC, F], wpool, "ff1", bufs=1, dt=F8A)
                w2 = load_w(w["dff2"], l, [P, FC, D], wpool, "ff2", bufs=1, dt=F8A)
                yn1 = layer_norm(y, LT, LN_D1(l), out_dt=F8A, apool=apool)
                attention(yn1, yn1, LT, LT, wq, wk, wv, wo, y, mask=dmask)
                yn2 = layer_norm(y, LT, LN_D2(l), out_dt=F8A, apool=apool)
                attention(yn2, zt, LT, LS, wq, wk, wv, wo, y, mask=None)
                yn3 = layer_norm(y, LT, LN_D3(l), out_dt=F8A, apool=apool, tag="xn8")
                ffn(yn3, w1, w2, LT, y)
            if dbg:
                nc.sync.dma_start(out=dbg["dbg_y"][:], in_=y[:])

        # --------------- generator + log-softmax ---------------
        with tc.tile_pool(name="gen_l", bufs=1) as lpool, \
             tc.tile_pool(name="gen_w", bufs=3) as gwpool, \
             tc.tile_pool(name="gen_a", bufs=2) as gapool, \
             tc.tile_pool(name="gen_p", bufs=2, space="PSUM") as gpp:
            cur_apool = gapool
            yf = layer_norm(y, LT, LN_DECN, out_dt=F8A, apool=gapool, tag="xn8")
            logits = [lpool.tile([P, VS], BF16, tag=f"log{t}", name=f"logits{t}")
                      for t in range(LT // P)]
            vchunks = []
            vs = 0
            while vs < VS:
                n = min(GCH, VS - vs)
                vchunks.append((vs, n))
                vs += n
            nech = (VS + ECH - 1) // ECH
            accs = [gapool.tile([P, nech], F32, tag=f"acc{t}", name=f"acc{t}")
                    for t in range(LT // P)]
            exp_done = [0]
            adone = {t: [] for t in range(LT // P)}
            with nc.allow_low_precision(reason="fp8 generator"):
                for j, (vs, n) in enumerate(vchunks):
                    gw = gwpool.tile([P, DC, GCH], F8A, tag="gw")
                    nc.sync.dma_start(out=gw[:, :, 0:n], in_=genw[:, :, vs:vs + n])
                    for t in range(LT // P):
                        gps = gpp.tile([P, 2, GCH // 2], F32, tag="gps")
                        for half in range(2):
                            h0 = half * (GCH // 2)
                            hn = min(GCH // 2, n - h0)
                            if hn <= 0:
                                continue
                            for kc in range(0, DC, KSTEP):
                                nc.tensor.matmul(
                                    gps[:, half, 0:hn],
                                    yf[:, kc:kc + KSTEP, t * P:(t + 1) * P] if fp8 else yf[:, kc, t * P:(t + 1) * P],
                                    gw[:, kc:kc + KSTEP, h0:h0 + hn] if fp8 else gw[:, kc, h0:h0 + hn],
                                    start=(kc == 0), stop=(kc == DC - KSTEP),
                                    perf_mode=PMODE)
                        if n == GCH:
                            dst = logits[t][:, vs:vs + n].rearrange(
                                "p (a b) -> p a b", a=2)
                            src_ap = gps[:]
                        else:
                            dst = logits[t][:, vs:vs + n]
                            src_ap = gps[:, 0, 0:n]
                        if (j + t) % 2 == 0:
                            nc.scalar.activation(out=dst, in_=src_ap,
                                                 func=AF.Identity, bias=0.0, scale=DSC)
                        else:
                            nc.vector.tensor_scalar_mul(dst, src_ap, DSC)
                    # fire exp/accum for any newly completed ECH-sized block
                    done = vs + n
                    while done - exp_done[0] >= ECH or (done == VS and exp_done[0] < VS):
                        es = exp_done[0]
                        n2 = min(ECH, VS - es)
                        for t in range(LT // P):
                            scr = gapool.tile([P, ECH], BF16, tag="scr", bufs=2)
                            nc.scalar.activation(out=scr[:, 0:n2],
                                                 in_=logits[t][:, es:es + n2],
                                                 func=AF.Exp, bias=0.0, scale=1.0,
                                                 accum_out=accs[t][:, len(adone[t]):len(adone[t]) + 1])
                            adone[t].append(es)
                        exp_done[0] += n2
            for t in range(LT // P):
                ssum = gapool.tile([P, 1], F32, tag="ssum")
                nc.vector.reduce_sum(ssum[:], accs[t][:], AX.X)
                logs = gapool.tile([P, 1], F32, tag="logs")
                nc.scalar.activation(out=logs[:], in_=ssum[:], func=AF.Ln,
                                     bias=eps_t[:], scale=1.0)
                nlog = gapool.tile([P, 1], F32, tag="nlog")
                nc.vector.tensor_scalar_mul(nlog[:], logs[:], -1.0)
                Q = VS // 8
                for q in range(8):
                    sl = slice(q * Q, (q + 1) * Q)
                    if q % 2 == 0:
                        nc.vector.tensor_scalar_sub(logits[t][:, sl], logits[t][:, sl], logs[:])
                    else:
                        nc.scalar.activation(out=logits[t][:, sl], in_=logits[t][:, sl],
                                             func=AF.Identity, bias=nlog[:], scale=1.0)
                    nc.sync.dma_start(out=out_d[t * P:(t + 1) * P, sl], in_=logits[t][:, sl])


# ---------------------------------------------------------------------------
# host side
# ---------------------------------------------------------------------------
def _pe_vec(bs):
    pos = np.arange(bs, dtype=np.float32)[:, None]
    div = np.exp(np.arange(0, D, 2, dtype=np.float32) * (-math.log(10000.0) / D))
    ang = pos * div
    return np.stack([np.sin(ang), np.cos(ang)], axis=-1).reshape(bs, D)


def _blk_w(wm, dt=ml_dtypes.bfloat16, scale=1.0):
    """[Din, Dout] -> [P, KC, Dout] with w[p, kc, n] = W[kc*128+p, n]."""
    din, dout = wm.shape
    kc = din // P
    a = wm.astype(np.float32) * scale
    return np.ascontiguousarray(a.reshape(kc, P, dout).transpose(1, 0, 2)).astype(dt)


def _blk_wo(wm, dt=ml_dtypes.bfloat16, scale=1.0):
    """Wo [NH*HD, D] -> head-pair packed [P, DC, D]:
    partition p=(h%2)*64+d, chunk c=h//2 holds Wo row h*64+d."""
    out = np.empty((P, DC, D), dtype=np.float32)
    for h in range(NH):
        rows = wm[h * HD:(h + 1) * HD, :] * scale
        out[(h % 2) * HD:(h % 2) * HD + HD, h // 2, :] = rows
    return np.ascontiguousarray(out).astype(dt)


def _blk_xT(xm, dt=ml_dtypes.bfloat16):
    """[T, D] -> transposed blocked [P, DC, T]."""
    t = xm.T  # [D, T]
    return np.ascontiguousarray(
        t.reshape(DC, P, xm.shape[0]).transpose(1, 0, 2)).astype(dt)


def kernel(**inputs):
    global LAST_RESULTS
    inp = {k: np.asarray(v) for k, v in inputs.items()}

    pe = _pe_vec(BS)
    x0 = inp["src_emb"].astype(np.float32)[inp["src"].astype(np.int64)] + pe[:, None, :]
    y0 = inp["tgt_emb"].astype(np.float32)[inp["tgt"].astype(np.int64)] + pe[:, None, :]

    msk_src = inp["msk_src"]
    msk_tgt = inp["msk_tgt"]
    assert np.all(msk_src != 0), "kernel assumes msk_src has no zeros"
    use_dec_mask = not np.all(msk_tgt != 0)

    # shared (replicated) weight tensors
    fp8 = bool(int(os.environ.get("KERNEL_FP8", "1")))
    pool = bool(int(os.environ.get("KERNEL_POOL", "1")))
    w8dt = ml_dtypes.float8_e4m3 if fp8 else ml_dtypes.bfloat16
    w8scale = FP8_SCALE if fp8 else 1.0
    shared = {}
    # wv output columns permuted head-parity-blocked: [0,2,4,6,1,3,5,7]
    vperm = np.concatenate([np.arange(h * HD, (h + 1) * HD)
                            for h in (0, 2, 4, 6, 1, 3, 5, 7)])
    for pfx in ("e", "d"):
        for nm in ("wq", "wk"):
            shared[pfx + nm] = np.stack([
                _blk_w(inp[pfx + nm + "_w"][l], dt=w8dt, scale=w8scale) for l in range(NL)])
        shared[pfx + "wv"] = np.stack([
            _blk_w(inp[pfx + "wv_w"][l][:, vperm], dt=w8dt, scale=w8scale)
            for l in range(NL)])
        for nm in ("ff1", "ff2"):
            shared[pfx + nm] = np.stack([
                _blk_w(inp[pfx + nm + "_w"][l], dt=w8dt, scale=w8scale) for l in range(NL)])
        shared[pfx + "wo"] = np.stack([
            _blk_wo(inp[pfx + "wo_w"][l], dt=w8dt, scale=w8scale) for l in range(NL)])
    shared["genw"] = _blk_w(inp["gen_w"], dt=w8dt, scale=w8scale)
    # scaled identities for residual-in-PSUM accumulation: [P, 2, P]
    eye = np.eye(P, dtype=np.float32)
    shared["ident"] = np.ascontiguousarray(np.stack(
        [eye * w8scale, eye * w8scale * w8scale],
        axis=1)).astype(ml_dtypes.bfloat16)

    for pfx in ("e", "d"):
        for nm in ("wq_b", "wk_b", "wv_b", "wo_b", "ff1_b", "ff2_b"):
            assert np.all(inp[pfx + nm] == 0), f"nonzero bias {pfx+nm} unsupported fast path"
    assert np.all(inp["gen_b"] == 0)

    # layer-norm params: ln_cols [P, 32, 2, DC], ln_rows [1, 32, D] fp16
    g_all = np.zeros((N_LN, D), np.float32)
    b_all = np.zeros((N_LN, D), np.float32)
    for l in range(NL):
        g_all[LN_E1(l)] = inp["eln1_g"][l]; b_all[LN_E1(l)] = inp["eln1_b"][l]
        g_all[LN_E2(l)] = inp["eln2_g"][l]; b_all[LN_E2(l)] = inp["eln2_b"][l]
        g_all[LN_D1(l)] = inp["dln1_g"][l]; b_all[LN_D1(l)] = inp["dln1_b"][l]
        g_all[LN_D2(l)] = inp["dln2_g"][l]; b_all[LN_D2(l)] = inp["dln2_b"][l]
        g_all[LN_D3(l)] = inp["dln3_g"][l]; b_all[LN_D3(l)] = inp["dln3_b"][l]
    g_all[LN_ENCN] = inp["encn_g"]; b_all[LN_ENCN] = inp["encn_b"]
    g_all[LN_DECN] = inp["decn_g"]; b_all[LN_DECN] = inp["decn_b"]
    ln_cols = np.stack([g_all, b_all], axis=1)          # [32, 2, D]
    ln_cols = ln_cols.reshape(N_LN, 2, DC, P).transpose(3, 0, 1, 2)  # [P, 32, 2, DC]
    shared["ln_cols"] = np.ascontiguousarray(ln_cols).astype(np.float32)
    shared["ln_rows"] = g_all.reshape(N_LN, 1, D).astype(np.float16)

    nc = build_program(use_dec_mask, debug=bool(int(os.environ.get("KERNEL_DEBUG", "0"))),
                       fp8=fp8, pool=pool)

    in_maps = []
    for b in range(BS):
        m = dict(shared)
        m["x0t"] = _blk_xT(x0[b])
        m["y0t"] = _blk_xT(y0[b])
        if use_dec_mask:
            mk = (msk_tgt[b].T != 0).astype(np.float32)  # [k, q]
            m["dmaskt"] = np.ascontiguousarray(
                mk.reshape(LT // P, P, LT).transpose(1, 0, 2)).astype(ml_dtypes.bfloat16)
        in_maps.append(m)

    res = run_bass_kernel_spmd(nc, in_maps, list(range(BS)))
    LAST_RESULTS = res
    out = np.stack([res.results[b]["out"].astype(np.float32) for b in range(BS)])
    return out
